# revision 31
# baseline (speedup 1.0000x reference)
"""GNN message-passing kernel for 8 Trainium2 NeuronCores.

Strategy
--------
Edges are sorted by receiver on the host and sharded across cores at
128-node "chunk" boundaries, so every core owns a disjoint contiguous node
range plus all edges pointing into it.  That removes the all-reduce
entirely: each core computes its own slice of the scattered segment-sums
and runs the node MLP for its own chunks.

On-device per 512-edge group (4 tiles of 128):
  * indirect-DMA gather of sender/receiver node rows, PE-transposed to
    feature-major [f, e] layout
  * edge features arrive pre-transposed from the host
  * 3-layer MLP as weight-stationary matmuls (fp32, N=512 free dim)
  * PE transpose back to [e, h], LayerNorm via fused DVE/ACT ops
  * segment-sum as one-hot matmuls accumulating into PSUM, then added
    into a persistent per-core effect SBUF tensor (dynamic offset via a
    register loaded from a host-prepared meta table)
Outputs are written in sorted edge order; the host unpermutes.
"""

import sys

for _p in ("/opt/trn_rl_repo",):
    if _p not in sys.path:
        sys.path.insert(0, _p)

import numpy as np

H = 128
P = 128
EG = 512          # edges per group
TPG = EG // P     # tiles per group
LN_EPS = 1e-5
NCORES = 8
W_NT = 2          # segment-sum windows per normal/tangential group (degree ~8)
W_D = 6           # windows per damp group (degree ~1 → a group spans ~EG nodes)


# ----------------------------------------------------------------------------
# host-side prep
# ----------------------------------------------------------------------------

def _edge_tables(rcv_sorted, snd_sorted, chunk0, n_groups, n_windows, n_chunks):
    """Per-core index/meta tables for one edge set (already padded to G*EG).

    rcv_sorted/snd_sorted: int64/int32 arrays of length n_groups*EG with -1
    padding rows.  chunk0: first global chunk owned by this core.
    Returns (sidx, ridx, loc, meta) with
      sidx/ridx [P, G*TPG] int32    gather row ids (0 for padding)
      loc       [P, G*TPG] float32  receiver - 128*c1(g)  (-1 for padding)
      meta      [1, W*G]   int32    element offsets of the window columns in
                                    the effect tensor free dim (clamped to
                                    the spare column when past the range)
    """
    G, W = n_groups, n_windows
    sidx = np.zeros((P, G * TPG), np.int32)
    ridx = np.zeros((P, G * TPG), np.int32)
    loc = np.full((P, G * TPG), -1.0, np.float32)
    meta = np.zeros((P, W * G), np.int32)
    ar = np.arange(P, dtype=np.int32)
    for g in range(G):
        seg = slice(g * EG, (g + 1) * EG)
        r = rcv_sorted[seg]
        s = snd_sorted[seg]
        real = r >= 0
        if real.any():
            lo = int(r[real][0])
            c1 = lo // P
        else:
            c1 = chunk0
        local_c1 = c1 - chunk0
        assert local_c1 >= 0
        for w in range(W):
            meta[:, W * g + w] = min(local_c1 + w, n_chunks) * P + ar
        lv = np.where(real, r - c1 * P, -1).astype(np.float32)
        if real.any():
            mx = lv[real].max()
            assert mx < W * P, f"group window span too large: {mx} (W={W})"
        sv = np.where(real, s, 0).astype(np.int32)
        rv = np.where(real, r, 0).astype(np.int32)
        sidx[:, g * TPG:(g + 1) * TPG] = sv.reshape(TPG, P).T
        ridx[:, g * TPG:(g + 1) * TPG] = rv.reshape(TPG, P).T
        loc[:, g * TPG:(g + 1) * TPG] = lv.reshape(TPG, P).T
    return sidx, ridx, loc, meta


def _pad_sorted(perm, rcv, snd, lo, hi, G):
    """Slice [lo:hi) of the sorted edge list, pad to G*EG with -1 rows."""
    n = hi - lo
    r = np.full(G * EG, -1, np.int64)
    s = np.full(G * EG, -1, np.int64)
    p = np.full(G * EG, 0, np.int64)
    r[:n] = rcv[lo:hi]
    s[:n] = snd[lo:hi]
    p[:n] = perm[lo:hi]
    return r, s, p, n


def _featT(feat, perm_padded, n_real):
    """[P, len*?] pre-transposed, permuted edge features (zeros padding)."""
    out = np.zeros((feat.shape[1], len(perm_padded)), np.float32)
    if n_real:
        out[:, :n_real] = feat[perm_padded[:n_real]].T
    return np.ascontiguousarray(out)


def prep_inputs(inputs, ncores=NCORES):
    """Split the full problem into per-core in_maps plus assembly metadata."""
    node = np.asarray(inputs["node"], np.float32)
    ne = np.asarray(inputs["normal_edge"], np.float32)
    te = np.asarray(inputs["tangential_edge"], np.float32)
    de = np.asarray(inputs["damping_edge"], np.float32)
    snd = np.asarray(inputs["senders"]).astype(np.int64)
    rcv = np.asarray(inputs["receivers"]).astype(np.int64)
    dsnd = np.asarray(inputs["self_edge_senders"]).astype(np.int64)
    drcv = np.asarray(inputs["self_edge_receivers"]).astype(np.int64)

    n_nodes = node.shape[0]
    n_chunks = -(-n_nodes // P)

    perm = np.argsort(rcv, kind="stable")
    rs = rcv[perm]
    ss = snd[perm]
    dperm = np.argsort(drcv, kind="stable")
    drs = drcv[dperm]
    dss = dsnd[dperm]

    cnt = np.bincount(rs // P, minlength=n_chunks)
    dcnt = np.bincount(drs // P, minlength=n_chunks)
    w = 2 * cnt + dcnt + 256          # rough per-chunk cost model
    cw = np.cumsum(w)
    bounds = [0]
    for c in range(1, ncores):
        bounds.append(int(np.searchsorted(cw, cw[-1] * c / ncores)) + 1)
    bounds.append(n_chunks)
    bounds = sorted(set(bounds))
    while len(bounds) < ncores + 1:    # degenerate; shouldn't happen
        bounds.append(n_chunks)
    K = max(bounds[c + 1] - bounds[c] for c in range(ncores))

    # per-core edge ranges (searchsorted on node-id boundaries)
    e_lo = [int(np.searchsorted(rs, bounds[c] * P)) for c in range(ncores)]
    e_lo.append(len(rs))
    d_lo = [int(np.searchsorted(drs, bounds[c] * P)) for c in range(ncores)]
    d_lo.append(len(drs))
    G = max(-(-(e_lo[c + 1] - e_lo[c]) // EG) for c in range(ncores))
    GD = max(1, max(-(-(d_lo[c + 1] - d_lo[c]) // EG) for c in range(ncores)))

    # constants shared by all cores
    iota = np.tile(np.arange(P, dtype=np.float32), (P, 1))
    w_max = max(W_NT, W_D)
    consts = {
        "iotas": np.concatenate([iota + P * w for w in range(w_max)], axis=1),
        "ident": np.eye(P, dtype=np.float32),
    }
    for nm in ("ne", "te", "de", "nm"):
        consts[f"{nm}_g_rep"] = np.tile(np.asarray(inputs[f"{nm}_g"], np.float32), (P, 1))
        consts[f"{nm}_bt_rep"] = np.tile(np.asarray(inputs[f"{nm}_bt"], np.float32), (P, 1))
        for b in ("b1", "b2", "b3"):
            consts[f"{nm}_{b}c"] = np.asarray(inputs[f"{nm}_{b}"], np.float32)[:, None].copy()
        for wn in ("W1", "W2", "W3"):
            consts[f"{nm}_{wn}"] = np.asarray(inputs[f"{nm}_{wn}"], np.float32)
    consts["epsc"] = np.full((P, 1), LN_EPS, np.float32)

    in_maps = []
    asm = {"perm": perm, "dperm": dperm, "bounds": bounds, "cfg": None,
           "e_n": [], "d_n": [], "node_rows": []}
    for c in range(ncores):
        chunk0 = bounds[c]
        kc = bounds[c + 1] - bounds[c]
        r_p, s_p, p_p, n_e = _pad_sorted(perm, rs, ss, e_lo[c], e_lo[c + 1], G)
        dr_p, ds_p, dp_p, n_d = _pad_sorted(dperm, drs, dss, d_lo[c], d_lo[c + 1], GD)
        sidx, ridx, loc, meta = _edge_tables(r_p, s_p, chunk0, G, W_NT, kc)
        dsidx, dridx, dloc, dmeta = _edge_tables(dr_p, ds_p, chunk0, GD, W_D, kc)

        nodeT = np.zeros((P, K * P), np.float32)
        row0 = chunk0 * P
        row1 = min(bounds[c + 1] * P, n_nodes)
        nodeT[:, :row1 - row0] = node[row0:row1].T

        m = {
            "node": node,
            "nodeT": nodeT,
            "neT": _featT(ne, p_p, n_e),
            "teT": _featT(te, p_p, n_e),
            "daT": _featT(de, dp_p, n_d),
            "sidx": sidx, "ridx": ridx, "loc": loc, "meta": meta,
            "dsidx": dsidx, "dridx": dridx, "dloc": dloc, "dmeta": dmeta,
        }
        m.update(consts)
        in_maps.append(m)
        asm["e_n"].append(n_e)
        asm["d_n"].append(n_d)
        asm["node_rows"].append((row0, row1))

    asm["cfg"] = dict(G=G, GD=GD, K=K, n_nodes=n_nodes)
    return in_maps, asm


# ----------------------------------------------------------------------------
# device program
# ----------------------------------------------------------------------------

def build_program(G, GD, K, n_nodes):
    import concourse.bacc as bacc
    import concourse.mybir as mybir
    import concourse.tile as tile
    import concourse.bass as bass

    f32 = mybir.dt.float32
    i32 = mybir.dt.int32
    Alu = mybir.AluOpType
    Act = mybir.ActivationFunctionType

    nc = bacc.Bacc(name="gnn_mp")

    dram = {}
    def din(name, shape, dtype=f32):
        dram[name] = nc.dram_tensor(name, list(shape), dtype, kind="ExternalInput")
        return dram[name]
    def dout(name, shape, dtype=f32):
        dram[name] = nc.dram_tensor(name, list(shape), dtype, kind="ExternalOutput")
        return dram[name]

    node_d = din("node", [n_nodes, H])
    nodeT_d = din("nodeT", [P, K * P])
    neT_d = din("neT", [P, G * EG])
    teT_d = din("teT", [P, G * EG])
    daT_d = din("daT", [P, GD * EG])
    sidx_d = din("sidx", [P, G * TPG], i32)
    ridx_d = din("ridx", [P, G * TPG], i32)
    loc_d = din("loc", [P, G * TPG])
    meta_d = din("meta", [P, W_NT * G], i32)
    dsidx_d = din("dsidx", [P, GD * TPG], i32)
    dridx_d = din("dridx", [P, GD * TPG], i32)
    dloc_d = din("dloc", [P, GD * TPG])
    dmeta_d = din("dmeta", [P, W_D * GD], i32)
    w_max = max(W_NT, W_D)
    iotas_d = din("iotas", [P, w_max * P])
    ident_d = din("ident", [P, P])
    epsc_d = din("epsc", [P, 1])
    wparams = {}
    for nm in ("ne", "te", "de", "nm"):
        kdim = 2 * H if nm == "nm" else 3 * H
        wparams[nm] = dict(
            W1=din(f"{nm}_W1", [kdim, H]), W2=din(f"{nm}_W2", [H, H]),
            W3=din(f"{nm}_W3", [H, H]),
            b1=din(f"{nm}_b1c", [P, 1]), b2=din(f"{nm}_b2c", [P, 1]),
            b3=din(f"{nm}_b3c", [P, 1]),
            g=din(f"{nm}_g_rep", [P, P]), bt=din(f"{nm}_bt_rep", [P, P]),
        )
    outn_d = dout("out_n", [G * EG, H])
    outt_d = dout("out_t", [G * EG, H])
    outd_d = dout("out_d", [GD * EG, H])
    outv_d = dout("out_v", [K * P, H])
    # segment-sum accumulator; PJRT zero-donates ExternalOutput buffers, so
    # it starts at zero and indirect scatter-DMAs accumulate into it.
    effect_d = dout("effect", [(K + 1) * P, H])

    with tile.TileContext(nc) as tc:
        from contextlib import ExitStack
        ctx = ExitStack()
        cpool = ctx.enter_context(tc.tile_pool(name="consts", bufs=1))
        wk = ctx.enter_context(tc.tile_pool(name="work", bufs=2))
        st = ctx.enter_context(tc.tile_pool(name="stats", bufs=2))
        ps = ctx.enter_context(tc.tile_pool(name="psT", bufs=2, space="PSUM"))
        psm = ctx.enter_context(tc.tile_pool(name="psM", bufs=2, space="PSUM"))
        pseg = ctx.enter_context(tc.tile_pool(name="psSeg", bufs=1, space="PSUM"))

        _const_n = [0]

        def const_tile(shape, src_ap, dtype=f32):
            _const_n[0] += 1
            t = cpool.tile(list(shape), dtype, tag=f"const{_const_n[0]}")
            nc.sync.dma_start(out=t[:], in_=src_ap)
            return t

        iotas_c = const_tile([P, w_max * P], iotas_d[:, :])
        ident_c = const_tile([P, P], ident_d[:, :])
        eps_c = const_tile([P, 1], epsc_d[:, :])

        W = {}
        for nm in ("ne", "te", "de", "nm"):
            p = wparams[nm]
            kdim = 2 if nm == "nm" else 3
            W[nm] = dict(
                W1=[const_tile([P, H], p["W1"][k * P:(k + 1) * P, :]) for k in range(kdim)],
                W2=const_tile([P, H], p["W2"][:, :]),
                W3=const_tile([P, H], p["W3"][:, :]),
                b1=const_tile([P, 1], p["b1"][:, :]),
                b2=const_tile([P, 1], p["b2"][:, :]),
                b3=const_tile([P, 1], p["b3"][:, :]),
                g=const_tile([P, P], p["g"][:, :]),
                bt=const_tile([P, P], p["bt"][:, :]),
            )

        def layer_norm(xT_ps, t_sb, y_sb, junk, mus, ssq, cols, mlp):
            """xT_ps: PSUM [P, n*P] of n transposed [e,h] tiles.
            Evacuates to t_sb and collects per-edge sum / sum-of-squares."""
            for j, col in enumerate(cols):
                sl = (slice(None), slice(j * P, (j + 1) * P))
                nc.vector.tensor_scalar(
                    out=t_sb[sl], in0=xT_ps[sl], scalar1=1.0, scalar2=None,
                    op0=Alu.mult, op1=Alu.add, accum_out=mus[:, col:col + 1])
                nc.scalar.activation(
                    out=junk[sl], in_=t_sb[sl], func=Act.Square,
                    accum_out=ssq[:, col:col + 1])

        def ln_finish(mus, ssq, ncols):
            mu = st.tile([P, ncols], f32, tag="mu")
            m2 = st.tile([P, ncols], f32, tag="m2")
            varr = st.tile([P, ncols], f32, tag="varr")
            sd = st.tile([P, ncols], f32, tag="sd")
            a = st.tile([P, ncols], f32, tag="a")
            b = st.tile([P, ncols], f32, tag="b")
            nc.vector.tensor_scalar(out=mu[:], in0=mus[:, :ncols], scalar1=1.0 / H,
                                    scalar2=None, op0=Alu.mult)
            nc.vector.tensor_tensor(out=m2[:], in0=mu[:], in1=mu[:], op=Alu.mult)
            nc.vector.scalar_tensor_tensor(out=varr[:], in0=ssq[:, :ncols],
                                           scalar=1.0 / H, in1=m2[:],
                                           op0=Alu.mult, op1=Alu.subtract)
            nc.scalar.activation(out=sd[:], in_=varr[:], func=Act.Sqrt,
                                 bias=eps_c[:, :], scale=1.0)
            nc.vector.reciprocal(out=a[:], in_=sd[:])
            nc.vector.scalar_tensor_tensor(out=b[:], in0=mu[:], scalar=-1.0,
                                           in1=a[:], op0=Alu.mult, op1=Alu.mult)
            return a, b

        def ln_apply(t_sb, y_sb, a, b, cols, mlp):
            for j, col in enumerate(cols):
                sl = (slice(None), slice(j * P, (j + 1) * P))
                B = wk.tile([P, P], f32, tag="lnB")
                nc.vector.scalar_tensor_tensor(
                    out=B[:], in0=W[mlp]["g"][:], scalar=b[:, col:col + 1],
                    in1=W[mlp]["bt"][:], op0=Alu.mult, op1=Alu.add)
                nc.vector.scalar_tensor_tensor(
                    out=y_sb[sl], in0=t_sb[sl], scalar=a[:, col:col + 1],
                    in1=B[:], op0=Alu.mult, op1=Alu.add)

        def mlp3(mlp, rhs_chunks, n_free):
            """rhs_chunks: list of SBUF [P, n_free] K-chunks. Returns x3 SBUF
            [P, n_free] (transposed layout, pre-LN, bias applied)."""
            wd = W[mlp]
            h1p = psm.tile([P, n_free], f32, tag="mmp")
            for k, rc in enumerate(rhs_chunks):
                nc.tensor.matmul(out=h1p[:], lhsT=wd["W1"][k][:], rhs=rc,
                                 start=(k == 0), stop=(k == len(rhs_chunks) - 1))
            h1 = wk.tile([P, n_free], f32, tag="h1")
            nc.scalar.activation(out=h1[:], in_=h1p[:], func=Act.Relu,
                                 bias=wd["b1"][:, :], scale=1.0)
            h2p = psm.tile([P, n_free], f32, tag="mmp")
            nc.tensor.matmul(out=h2p[:], lhsT=wd["W2"][:], rhs=h1[:],
                             start=True, stop=True)
            h2 = wk.tile([P, n_free], f32, tag="h2")
            nc.scalar.activation(out=h2[:], in_=h2p[:], func=Act.Relu,
                                 bias=wd["b2"][:, :], scale=1.0)
            x3p = psm.tile([P, n_free], f32, tag="mmp")
            nc.tensor.matmul(out=x3p[:], lhsT=wd["W3"][:], rhs=h2[:],
                             start=True, stop=True)
            x3 = wk.tile([P, n_free], f32, tag="x3")
            nc.scalar.activation(out=x3[:], in_=x3p[:], func=Act.Identity,
                                 bias=wd["b3"][:, :], scale=1.0)
            return x3

        def transpose_batch(src_sb, ntiles, tag):
            """PE-transpose ntiles [P,P] tiles of src_sb into one PSUM bank."""
            tp = ps.tile([P, ntiles * P], f32, tag="tp")
            for t in range(ntiles):
                sl = (slice(None), slice(t * P, (t + 1) * P))
                nc.tensor.transpose(out=tp[sl], in_=src_sb[sl], identity=ident_c[:])
            return tp

        def gather_T(idx_sb, t_range, tag):
            """Indirect-gather node rows for each tile and transpose to [f,e]."""
            rows = wk.tile([P, len(t_range) * P], f32, tag=f"{tag}_rows")
            for j, t in enumerate(t_range):
                nc.gpsimd.indirect_dma_start(
                    out=rows[:, j * P:(j + 1) * P], out_offset=None,
                    in_=node_d[:, :],
                    in_offset=bass.IndirectOffsetOnAxis(ap=idx_sb[:, t:t + 1], axis=0),
                )
            tp = transpose_batch(rows, len(t_range), tag=f"{tag}_tp")
            sb = wk.tile([P, len(t_range) * P], f32, tag=f"{tag}_sb")
            nc.scalar.activation(out=sb[:], in_=tp[:], func=Act.Copy, bias=0.0)
            return sb

        def edge_group(g, mlps, featT_ds, sidx_ds, ridx_ds, loc_ds, meta_ds,
                       out_ds, tag, n_windows):
            """One 512-edge group.  mlps: list of mlp names (1 or 2)."""
            sidx = wk.tile([P, TPG], i32, tag="sidx")
            nc.sync.dma_start(out=sidx[:], in_=sidx_ds[:, g * TPG:(g + 1) * TPG])
            ridx = wk.tile([P, TPG], i32, tag="ridx")
            nc.sync.dma_start(out=ridx[:], in_=ridx_ds[:, g * TPG:(g + 1) * TPG])
            locs = wk.tile([P, TPG], f32, tag="locs")
            nc.sync.dma_start(out=locs[:], in_=loc_ds[:, g * TPG:(g + 1) * TPG])

            sT = gather_T(sidx, range(TPG), "sT")
            rT = gather_T(ridx, range(TPG), "rT")

            mus = st.tile([P, 8], f32, tag="mus")
            ssq = st.tile([P, 8], f32, tag="ssq")
            ys = []
            for mi, mlp in enumerate(mlps):
                feat = wk.tile([P, EG], f32, tag=f"feat{mi}")
                nc.sync.dma_start(out=feat[:], in_=featT_ds[mi][:, g * EG:(g + 1) * EG])
                x3 = mlp3(mlp, [sT[:], rT[:], feat[:]], EG)
                xT = transpose_batch(x3, TPG, tag="xT")
                t_sb = wk.tile([P, EG], f32, tag=f"t{mi}")
                junk = wk.tile([P, EG], f32, tag=f"junk{mi}")
                y_sb = wk.tile([P, EG], f32, tag=f"y{mi}")
                cols = [mi * TPG + t for t in range(TPG)]
                layer_norm(xT, t_sb, y_sb, junk, mus, ssq, cols, mlp)
                ys.append((mlp, t_sb, y_sb, cols))
            a, b = ln_finish(mus, ssq, TPG * len(mlps))
            for (mlp, t_sb, y_sb, cols), od in zip(ys, out_ds):
                ln_apply(t_sb, y_sb, a, b, cols, mlp)
                nc.sync.dma_start(
                    out=od[g * EG:(g + 1) * EG, :].rearrange("(t p) h -> p t h", p=P),
                    in_=y_sb[:].rearrange("p (t h) -> p t h", t=TPG),
                )

            # segment sum: z = sum of type outputs, one-hot matmuls per window
            if len(ys) == 2:
                z = wk.tile([P, EG], f32, tag="z")
                nc.vector.tensor_tensor(out=z[:], in0=ys[0][2][:], in1=ys[1][2][:],
                                        op=Alu.add)
            else:
                z = ys[0][2]
            widx = wk.tile([P, n_windows], i32, tag="widx")
            nc.sync.dma_start(out=widx[:],
                              in_=meta_ds[:, n_windows * g:n_windows * (g + 1)])
            for w in range(n_windows):
                seg = pseg.tile([P, P], f32, tag=f"seg{w % 2}")
                for t in range(TPG):
                    S1 = wk.tile([P, P], f32, tag=f"S{w % 2}")
                    nc.vector.tensor_scalar(
                        out=S1[:], in0=iotas_c[:, w * P:(w + 1) * P],
                        scalar1=locs[:, t:t + 1], scalar2=None, op0=Alu.is_equal)
                    nc.tensor.matmul(out=seg[:], lhsT=S1[:], rhs=z[:, t * P:(t + 1) * P],
                                     start=(t == 0), stop=(t == TPG - 1))
                ev = wk.tile([P, P], f32, tag=f"ev{w % 2}")
                nc.vector.tensor_scalar(out=ev[:], in0=seg[:], scalar1=1.0,
                                        scalar2=None, op0=Alu.mult)
                nc.gpsimd.indirect_dma_start(
                    out=effect_d[:, :],
                    out_offset=bass.IndirectOffsetOnAxis(ap=widx[:, w:w + 1], axis=0),
                    in_=ev[:], in_offset=None, compute_op=Alu.add)

        # --- damp groups, then normal/tangential groups ---
        for g in range(GD):
            edge_group(g, ["de"], [daT_d], dsidx_d, dridx_d, dloc_d, dmeta_d,
                       [outd_d], "d", W_D)
        for g in range(G):
            edge_group(g, ["ne", "te"], [neT_d, teT_d], sidx_d, ridx_d, loc_d,
                       meta_d, [outn_d, outt_d], "e", W_NT)

        # --- node MLP per chunk ---
        for k in range(K):
            eff_sb = wk.tile([P, P], f32, tag="eff_sb")
            nc.sync.dma_start(out=eff_sb[:], in_=effect_d[k * P:(k + 1) * P, :])
            effTp = transpose_batch(eff_sb, 1, tag="effT")
            effT = wk.tile([P, P], f32, tag="effT_sb")
            nc.scalar.activation(out=effT[:], in_=effTp[:], func=Act.Copy, bias=0.0)
            nT = wk.tile([P, P], f32, tag="nT")
            nc.sync.dma_start(out=nT[:], in_=nodeT_d[:, k * P:(k + 1) * P])
            x3 = mlp3("nm", [effT[:], nT[:]], P)
            xT = transpose_batch(x3, 1, tag="xT")
            mus = st.tile([P, 8], f32, tag="mus")
            ssq = st.tile([P, 8], f32, tag="ssq")
            t_sb = wk.tile([P, P], f32, tag="t0")
            junk = wk.tile([P, P], f32, tag="junk0")
            y_sb = wk.tile([P, P], f32, tag="y0")
            layer_norm(xT, t_sb, y_sb, junk, mus, ssq, [0], "nm")
            a, b = ln_finish(mus, ssq, 1)
            ln_apply(t_sb, y_sb, a, b, [0], "nm")
            nc.sync.dma_start(out=outv_d[k * P:(k + 1) * P, :], in_=y_sb[:])

        ctx.close()

    nc.compile()
    return nc


# ----------------------------------------------------------------------------
# top-level entry
# ----------------------------------------------------------------------------

def kernel(**inputs):
    from concourse.bass_utils import run_bass_kernel_spmd

    in_maps, asm = prep_inputs(inputs, NCORES)
    cfg = asm["cfg"]
    nc = build_program(**cfg)
    core_ids = list(range(NCORES))
    res = run_bass_kernel_spmd(nc, in_maps, core_ids)
    return assemble(res.results, inputs, asm)


def assemble(results, inputs, asm):
    E = len(asm["perm"])
    S = len(asm["dperm"])
    n_nodes = asm["cfg"]["n_nodes"]
    out_normal = np.empty((E, H), np.float32)
    out_tang = np.empty((E, H), np.float32)
    out_damp = np.empty((S, H), np.float32)
    out_node = np.empty((n_nodes, H), np.float32)
    sn = np.concatenate([results[c]["out_n"][:asm["e_n"][c]] for c in range(len(results))])
    stg = np.concatenate([results[c]["out_t"][:asm["e_n"][c]] for c in range(len(results))])
    sd = np.concatenate([results[c]["out_d"][:asm["d_n"][c]] for c in range(len(results))])
    out_normal[asm["perm"]] = sn
    out_tang[asm["perm"]] = stg
    out_damp[asm["dperm"]] = sd
    for c, (r0, r1) in enumerate(asm["node_rows"]):
        out_node[r0:r1] = results[c]["out_v"][:r1 - r0]
    return out_node, out_normal, out_tang, out_damp
